# revision 16
# baseline (speedup 1.0000x reference)
"""Contrastive-learning loss kernel for Trainium2 (8 NeuronCores, Bass/Tile).

Problem (hardcoded shapes): B=16, L=512, DIN1=256, DIN2=192, DH=256, DF=128.
  emb1 = MLP_a(feature1); emb2 = MLP_b(feature2)          # (B, L, DF)
  positive = rowdot(f1, f2) + band-mean terms              # (N,)  N = B*L = 8192
  negative = logsumexp(f1 @ f2.T, axis=-1) - log N         # (N,)
  loss = mean(-positive + negative)

Sharding: data-parallel over B for embeddings/positives (2 batches per core);
the N x N negatives matrix is sharded row-wise. Each core computes the full
emb2 from a column-ROTATED copy of feature2 (its own batches first), so the
device program is identical across cores (pure SPMD, no partition-id): the
local rows are always columns [0, 1024) and logsumexp is invariant to column
order.

Schedule: the ScalarE exp (65536 elements/partition) is the hard floor
(~55us at 1 elem/cycle); everything else hides under it. All PSUM flows
through one 2-buffer (128, 2048) ring (8 banks), so every exp activation is
2048 wide with a fused row-sum (accum_out) and a fused per-row b2b bias
(exp(s + e1.b2b)); e2t itself stays bias-free. DMA descriptors are issued
from the GpSimd sequencer (cheap) in consumption order so MLP1 starts ~3us
and the first exp fires ~8us. MLP2 production, transposes, banded-positive
matmuls and the positive dot products are interleaved into PE/DVE/Pool slack
between negative chunks.

Outputs per core: pos_out (128, 8), se_out (128, 8) where column t holds
local rows [t*128, (t+1)*128). Host: loss = mean(-pos + log(se) - log N).
"""

import numpy as np

import concourse.bacc as bacc
import concourse.tile as tile
from concourse import mybir
from concourse.bass_utils import run_bass_kernel_spmd
from concourse.masks import make_identity

F32 = mybir.dt.float32
F32R = mybir.dt.float32r
BF16 = mybir.dt.bfloat16

B, L, DIN1, DIN2, DH, DF = 16, 512, 256, 192, 256, 128
N = B * L            # 8192 total rows
NCORES = 8
NB = B // NCORES     # 2 local batches per core
NLOC = NB * L        # 1024 local rows per core
NT = NLOC // 128     # 8 local row tiles
RING_FD = 2048       # ring tile free size (4 PSUM banks)
NGRP = N // RING_FD  # 4 negative column groups of 2048


def _build(share_tgt: bool):
    nc = bacc.Bacc("TRN2", target_bir_lowering=False, debug=False)

    x1t_d = nc.dram_tensor("x1t", [DIN1, NLOC], BF16, kind="ExternalInput")
    x2t_d = nc.dram_tensor("x2t", [DIN2, N], BF16, kind="ExternalInput")
    w1a_d = nc.dram_tensor("w1a", [DIN1, DH], BF16, kind="ExternalInput")
    w2a_d = nc.dram_tensor("w2a", [DH, DF], F32R, kind="ExternalInput")
    w1b_d = nc.dram_tensor("w1b", [DIN2, DH], BF16, kind="ExternalInput")
    w2b_d = nc.dram_tensor("w2b", [DH, DF], F32R, kind="ExternalInput")
    b1a_d = nc.dram_tensor("b1a", [128, 2], F32, kind="ExternalInput")
    b2a_d = nc.dram_tensor("b2a", [128, 1], F32, kind="ExternalInput")
    b1b_d = nc.dram_tensor("b1b", [128, 2], F32, kind="ExternalInput")
    b2b_d = nc.dram_tensor("b2b", [128, 1], F32, kind="ExternalInput")
    b2bt_d = nc.dram_tensor("b2bt", [128, DF], F32, kind="ExternalInput")
    bms_d = nc.dram_tensor("bms", [L, L], BF16, kind="ExternalInput")
    cis_d = nc.dram_tensor("cis", [128, NT], F32, kind="ExternalInput")
    if not share_tgt:
        bmt_d = nc.dram_tensor("bmt", [L, L], BF16, kind="ExternalInput")
        cit_d = nc.dram_tensor("cit", [128, NT], F32, kind="ExternalInput")
    pos_d = nc.dram_tensor("pos_out", [128, NT], F32, kind="ExternalOutput")
    se_d = nc.dram_tensor("se_out", [128, NT], F32, kind="ExternalOutput")

    with tile.TileContext(nc) as tc:
        import contextlib

        with contextlib.ExitStack() as stack:
            const = stack.enter_context(tc.tile_pool(name="const", bufs=1))
            big = stack.enter_context(tc.tile_pool(name="big", bufs=1))
            h2pool = stack.enter_context(tc.tile_pool(name="h2pool", bufs=3))
            posp = stack.enter_context(tc.tile_pool(name="posp", bufs=2))
            psum = stack.enter_context(tc.tile_pool(name="psum", bufs=1, space="PSUM"))

            # ---- SBUF tiles -------------------------------------------------
            w1a = const.tile([128, 2, DH], BF16)
            x1t = big.tile([128, 2, NLOC], BF16)
            w1b_a = const.tile([128, DH], BF16)
            w1b_b = const.tile([64, DH], BF16)
            x2a = big.tile([128, N], BF16)
            x2b = big.tile([64, N], BF16)
            w2a = const.tile([128, 2, DF], F32R)
            w2b = const.tile([128, 2, DF], F32R)
            b1a = const.tile([128, 2], F32)
            b2a = const.tile([128, 1], F32)
            b1b = const.tile([128, 2], F32)
            b2b = const.tile([128, 1], F32)
            b2bt = const.tile([128, DF], F32)
            bms = const.tile([128, 4, L], BF16)
            cis = const.tile([128, NT], F32)
            if share_tgt:
                bmt, cit = bms, cis
            else:
                bmt = const.tile([128, 4, L], BF16)
                cit = const.tile([128, NT], F32)

            # ---- DMA issues from the GpSimd sequencer (cheap), in
            # consumption order so MLP1 can start ~3us in.
            q = nc.gpsimd
            q.dma_start(out=w1a[:], in_=w1a_d.ap().rearrange("(t p) m -> p t m", p=128))
            for cc in range(2):
                q.dma_start(
                    out=x1t[:, :, cc * 512 : (cc + 1) * 512],
                    in_=x1t_d.ap().rearrange("(t p) c -> p t c", p=128)[
                        :, :, cc * 512 : (cc + 1) * 512
                    ],
                )
            q.dma_start(out=w1b_a[:], in_=w1b_d.ap()[0:128, :])
            q.dma_start(out=w1b_b[:], in_=w1b_d.ap()[128:DIN2, :])
            q.dma_start(out=x2a[:, 0:1024], in_=x2t_d.ap()[0:128, 0:1024])
            q.dma_start(out=x2b[:, 0:1024], in_=x2t_d.ap()[128:DIN2, 0:1024])
            q.dma_start(out=w2a[:], in_=w2a_d.ap().rearrange("(t p) m -> p t m", p=128))
            q.dma_start(out=w2b[:], in_=w2b_d.ap().rearrange("(t p) m -> p t m", p=128))
            q.dma_start(out=b1a[:], in_=b1a_d.ap())
            q.dma_start(out=b2a[:], in_=b2a_d.ap())
            q.dma_start(out=b1b[:], in_=b1b_d.ap())
            q.dma_start(out=b2b[:], in_=b2b_d.ap())
            q.dma_start(out=b2bt[:], in_=b2bt_d.ap())
            for g in range(1, 8):
                cs = slice(g * 1024, (g + 1) * 1024)
                q.dma_start(out=x2a[:, cs], in_=x2t_d.ap()[0:128, cs])
                q.dma_start(out=x2b[:, cs], in_=x2t_d.ap()[128:DIN2, cs])
            q.dma_start(out=bms[:], in_=bms_d.ap().rearrange("(t p) j -> p t j", p=128))
            q.dma_start(out=cis[:], in_=cis_d.ap())
            if not share_tgt:
                q.dma_start(
                    out=bmt[:], in_=bmt_d.ap().rearrange("(t p) j -> p t j", p=128)
                )
                q.dma_start(out=cit[:], in_=cit_d.ap())

            ident = const.tile([128, 128], F32)
            make_identity(nc, ident[:])
            zr_l = const.tile([128, 128], BF16)
            nc.gpsimd.memset(zr_l[:], 0.0)
            zr_r = const.tile([128, 512], BF16)
            nc.gpsimd.memset(zr_r[:], 0.0)

            e1t = big.tile([128, NLOC], F32R)
            e2t = big.tile([128, N], F32R)
            e2loc = big.tile([128, NLOC], F32R)
            h1t = big.tile([128, 2, NLOC], F32R)
            e1nat = big.tile([128, NT, DF], BF16)
            e2nat = big.tile([128, NT, DF], BF16)
            w1nat = big.tile([128, NT, DF], F32)
            w2snat = big.tile([128, NT, DF], F32)
            w2tnat = w2snat if share_tgt else big.tile([128, NT, DF], F32)
            rowbias = big.tile([128, NT], F32)
            pos_all = big.tile([128, NT], F32)
            acc_all = big.tile([128, NT * NGRP], F32)
            se_all = big.tile([128, NT], F32)

            def ring(name):
                return psum.tile([128, RING_FD], F32, tag="ring", bufs=2, name=name)

            # ---- PE warm-up: ramp the clock while input DMAs stream --------
            for w in range(2):
                wt = ring(f"warm{w}")
                for i in range(4):
                    nc.tensor.matmul(
                        wt[:, i * 512 : (i + 1) * 512], zr_l[:], zr_r[:],
                        start=True, stop=True,
                    )

            # ---- MLP1: h1 = relu(W1a^T x1 + b1a); e1 = W2a^T h1 + b2a ------
            h1ps = ring("h1ps")  # [cc, mt, 512]
            for cc in range(2):
                for mt in range(2):
                    sl = slice(cc * 1024 + mt * 512, cc * 1024 + (mt + 1) * 512)
                    for kt in range(2):
                        nc.tensor.matmul(
                            h1ps[:, sl],
                            w1a[:, kt, mt * 128 : (mt + 1) * 128],
                            x1t[:, kt, cc * 512 : (cc + 1) * 512],
                            start=(kt == 0),
                            stop=(kt == 1),
                        )
            h1v = h1ps[:].rearrange("p (cc mt c) -> p cc mt c", cc=2, mt=2)
            for mt in range(2):
                nc.vector.tensor_scalar(
                    out=h1t[:, mt, :].rearrange("p (cc c) -> p cc c", cc=2),
                    in0=h1v[:, :, mt, :],
                    scalar1=b1a[:, mt : mt + 1],
                    scalar2=0.0,
                    op0=mybir.AluOpType.add,
                    op1=mybir.AluOpType.max,
                )
            e1ps = ring("e1ps")
            for cc in range(2):
                for kt in range(2):
                    nc.tensor.matmul(
                        e1ps[:, cc * 512 : (cc + 1) * 512],
                        w2a[:, kt, :],
                        h1t[:, kt, cc * 512 : (cc + 1) * 512],
                        start=(kt == 0),
                        stop=(kt == 1),
                    )
            nc.vector.tensor_scalar_add(
                out=e1t[:], in0=e1ps[:, 0:NLOC], scalar1=b2a[:]
            )

            # e1-side transposes (tokens -> partitions), needs only e1t
            tp1 = ring("tp1")
            for t in range(NT):
                nc.tensor.transpose(
                    tp1[:, t * 128 : (t + 1) * 128],
                    e1t[:, t * 128 : (t + 1) * 128].bitcast(F32),
                    ident[:],
                )
            nc.vector.tensor_copy(
                e1nat[:], tp1[:, 0 : NT * 128].rearrange("p (t f) -> p t f", t=NT)
            )

            # rowbias[:, t] = e1_row . b2b  (fused into the exp activation)
            rbt = posp.tile([128, NT, DF], F32, tag="posg", name="rbt")
            for t in range(NT):
                nc.vector.tensor_mul(rbt[:, t, :], e1nat[:, t, :], b2bt[:])
            nc.vector.tensor_reduce(
                out=rowbias[:], in_=rbt[:], axis=mybir.AxisListType.X,
                op=mybir.AluOpType.add,
            )

            # ---- MLP2 production: one ring turn per pair of 512-col chunks
            def mlp2_pair(cp):
                c0 = cp * 1024
                h2ps = ring(f"h2ps{cp}")  # [cc, mt, 512]
                for cc in range(2):
                    cols = slice(c0 + cc * 512, c0 + (cc + 1) * 512)
                    for mt in range(2):
                        sl = slice(cc * 1024 + mt * 512, cc * 1024 + (mt + 1) * 512)
                        msl = slice(mt * 128, (mt + 1) * 128)
                        nc.tensor.matmul(
                            h2ps[:, sl], w1b_a[:, msl], x2a[:, cols],
                            start=True, stop=False,
                        )
                        nc.tensor.matmul(
                            h2ps[:, sl], w1b_b[:, msl], x2b[:, cols],
                            start=False, stop=True,
                        )
                h2t = h2pool.tile([128, 2, 1024], F32R, tag="h2t", name=f"h2t{cp}")
                h2v = h2ps[:].rearrange("p (cc mt c) -> p cc mt c", cc=2, mt=2)
                for mt in range(2):
                    nc.vector.tensor_scalar(
                        out=h2t[:, mt, :].rearrange("p (cc c) -> p cc c", cc=2),
                        in0=h2v[:, :, mt, :],
                        scalar1=b1b[:, mt : mt + 1],
                        scalar2=0.0,
                        op0=mybir.AluOpType.add,
                        op1=mybir.AluOpType.max,
                    )
                e2ps = ring(f"e2ps{cp}")
                for cc in range(2):
                    for kt in range(2):
                        nc.tensor.matmul(
                            e2ps[:, cc * 512 : (cc + 1) * 512],
                            w2b[:, kt, :],
                            h2t[:, kt, cc * 512 : (cc + 1) * 512],
                            start=(kt == 0),
                            stop=(kt == 1),
                        )
                nc.vector.tensor_copy(e2t[:, c0 : c0 + 1024], e2ps[:, 0:1024])

            mlp2_pair(0)  # local columns [0, 1024)

            # biased local e2 for positives/bands; e2t itself stays bias-free
            nc.vector.tensor_scalar_add(
                out=e2loc[:], in0=e2t[:, 0:NLOC], scalar1=b2b[:]
            )
            tp2 = ring("tp2")
            for t in range(NT):
                nc.tensor.transpose(
                    tp2[:, t * 128 : (t + 1) * 128],
                    e2loc[:, t * 128 : (t + 1) * 128].bitcast(F32),
                    ident[:],
                )
            nc.vector.tensor_copy(
                e2nat[:], tp2[:, 0 : NT * 128].rearrange("p (t f) -> p t f", t=NT)
            )

            mlp2_pair(1)

            # banded sums: one ring turn covers one batch (4 j-tiles, 512 cols)
            def band_batch(dst, bm, src, b):
                bp = ring(f"band{b}")
                for jt in range(4):
                    sl = slice(jt * 128, (jt + 1) * 128)
                    for mt in range(4):
                        nc.tensor.matmul(
                            bp[:, sl],
                            bm[:, mt, jt * 128 : (jt + 1) * 128],
                            src[:, 4 * b + mt, :],
                            start=(mt == 0),
                            stop=(mt == 3),
                        )
                nc.vector.tensor_copy(
                    dst[:, 4 * b : 4 * b + 4, :],
                    bp[:, 0:512].rearrange("p (s f) -> p s f", s=4),
                )

            # ---- negatives: one 2048-col exp per (row tile, col group) -----
            def neg_chunk(t, g):
                lhs = e1t[:, t * 128 : (t + 1) * 128]
                np_ps = ring(f"neg{t}_{g}")
                for i in range(RING_FD // 512):
                    c0 = g * RING_FD + i * 512
                    nc.tensor.matmul(
                        np_ps[:, i * 512 : (i + 1) * 512],
                        lhs,
                        e2t[:, c0 : c0 + 512],
                        start=True,
                        stop=True,
                    )
                idx = t * NGRP + g
                nc.scalar.activation(
                    out=np_ps[:],
                    in_=np_ps[:],
                    func=mybir.ActivationFunctionType.Exp,
                    bias=rowbias[:, t : t + 1],
                    accum_out=acc_all[:, idx : idx + 1],
                )

            # group 0 (cols 0:2048) with band turns as PE filler
            neg_chunk(0, 0)
            band_batch(w1nat, bms, e1nat, 0)
            neg_chunk(1, 0)
            band_batch(w1nat, bms, e1nat, 1)
            neg_chunk(2, 0)
            band_batch(w2snat, bms, e2nat, 0)
            neg_chunk(3, 0)
            band_batch(w2snat, bms, e2nat, 1)
            neg_chunk(4, 0)
            if not share_tgt:
                band_batch(w2tnat, bmt, e2nat, 0)
            neg_chunk(5, 0)
            if not share_tgt:
                band_batch(w2tnat, bmt, e2nat, 1)
            neg_chunk(6, 0)
            mlp2_pair(2)
            neg_chunk(7, 0)
            mlp2_pair(3)

            # group 1 (cols 2048:4096); produce cols 4096:8192 in the slack
            for t in range(NT):
                neg_chunk(t, 1)
                if t < 4:
                    mlp2_pair(4 + t)

            # pos = rowdot(e1,e2loc) + band-mean terms (all 8 tiles at once)
            ga = posp.tile([128, NT, DF], F32, tag="posg")
            r1 = posp.tile([128, NT], F32, tag="post")
            r2 = posp.tile([128, NT], F32, tag="post")
            if share_tgt:
                nc.vector.tensor_add(ga[:], w1nat[:], w2snat[:])
                nc.vector.tensor_mul(ga[:], ga[:], e1nat[:])
            else:
                nc.vector.tensor_mul(ga[:], w1nat[:], e1nat[:])
            nc.vector.tensor_reduce(
                out=r1[:], in_=ga[:], axis=mybir.AxisListType.X, op=mybir.AluOpType.add
            )
            gb = posp.tile([128, NT, DF], F32, tag="posg")
            nc.vector.tensor_mul(gb[:], w2snat[:], e2nat[:])
            nc.vector.tensor_reduce(
                out=r2[:], in_=gb[:], axis=mybir.AxisListType.X, op=mybir.AluOpType.add
            )
            nc.vector.tensor_add(r1[:], r1[:], r2[:])
            nc.vector.tensor_mul(r1[:], r1[:], cis[:])
            if not share_tgt:
                gc = posp.tile([128, NT, DF], F32, tag="posg")
                nc.vector.tensor_mul(gc[:], w2tnat[:], e1nat[:])
                rt = posp.tile([128, NT], F32, tag="post")
                nc.vector.tensor_reduce(
                    out=rt[:], in_=gc[:], axis=mybir.AxisListType.X,
                    op=mybir.AluOpType.add,
                )
                nc.vector.tensor_mul(rt[:], rt[:], cit[:])
                nc.vector.tensor_add(r1[:], r1[:], rt[:])
            gd = posp.tile([128, NT, DF], BF16, tag="posgb")
            nc.vector.tensor_mul(gd[:], e1nat[:], e2nat[:])
            r3 = posp.tile([128, NT], F32, tag="post")
            nc.vector.tensor_reduce(
                out=r3[:], in_=gd[:], axis=mybir.AxisListType.X, op=mybir.AluOpType.add
            )
            nc.vector.tensor_add(pos_all[:], r1[:], r3[:])
            nc.sync.dma_start(out=pos_d.ap(), in_=pos_all[:])

            # groups 2..3
            for g in range(2, NGRP):
                for t in range(NT):
                    neg_chunk(t, g)

            nc.vector.tensor_reduce(
                out=se_all[:],
                in_=acc_all[:].rearrange("p (t g) -> p t g", t=NT),
                axis=mybir.AxisListType.X,
                op=mybir.AluOpType.add,
            )
            nc.sync.dma_start(out=se_d.ap(), in_=se_all[:])

    nc.compile()
    return nc


_BUILD_CACHE: dict = {}


def _get_nc(share_tgt: bool):
    if share_tgt not in _BUILD_CACHE:
        _BUILD_CACHE[share_tgt] = _build(share_tgt)
    return _BUILD_CACHE[share_tgt]


def _band_mask(r: int) -> np.ndarray:
    """mask[m, j] = 1 if |m-j| <= r (and inside [0,L)) else 0."""
    bm = np.zeros((L, L), dtype=np.float32)
    if r > 0:
        j = np.arange(L)
        lo = np.maximum(j - r, 0)
        hi = np.minimum(j + r + 1, L)
        m = np.arange(L)[:, None]
        bm = ((m >= lo[None, :]) & (m < hi[None, :])).astype(np.float32)
    return bm


def _cnt_inv(r: int) -> np.ndarray:
    """(128, NT) tile of 1/count(j) per local row (j = row mod L)."""
    j = np.arange(L)
    if r > 0:
        cnt = (np.minimum(j + r + 1, L) - np.maximum(j - r, 0)).astype(np.float64)
    else:
        cnt = np.ones(L)
    cinv = (1.0 / cnt).astype(np.float32)
    rows = (np.arange(NLOC) % L)
    return np.ascontiguousarray(cinv[rows].reshape(NT, 128).T)


def kernel(**inputs):
    loss, _ = _run(inputs, trace=False)
    return loss


def _run(inputs, trace=False, trace_kwargs=None):
    import ml_dtypes

    bf16 = ml_dtypes.bfloat16
    feature1 = inputs["feature1"]
    feature2 = inputs["feature2"]
    W1a, b1a, W2a, b2a = inputs["W1a"], inputs["b1a"], inputs["W2a"], inputs["b2a"]
    W1b, b1b, W2b, b2b = inputs["W1b"], inputs["b1b"], inputs["W2b"], inputs["b2b"]
    f1 = np.ascontiguousarray(np.asarray(feature1, dtype=np.float32))
    f2 = np.ascontiguousarray(np.asarray(feature2, dtype=np.float32))
    r_self = int(np.asarray(inputs["positive_range_self"]))
    r_tgt = int(np.asarray(inputs["positive_range_tgt"]))
    share_tgt = r_tgt == r_self

    nc = _get_nc(share_tgt)

    x2t_full = np.ascontiguousarray(f2.reshape(N, DIN2).T.astype(bf16))  # (192, 8192)
    common = {
        "w1a": np.ascontiguousarray(np.asarray(W1a, np.float32).astype(bf16)),
        "w2a": np.ascontiguousarray(np.asarray(W2a, np.float32)),
        "w1b": np.ascontiguousarray(np.asarray(W1b, np.float32).astype(bf16)),
        "w2b": np.ascontiguousarray(np.asarray(W2b, np.float32)),
        "b1a": np.ascontiguousarray(np.asarray(b1a, np.float32).reshape(2, 128).T),
        "b2a": np.asarray(b2a, np.float32).reshape(128, 1),
        "b1b": np.ascontiguousarray(np.asarray(b1b, np.float32).reshape(2, 128).T),
        "b2b": np.asarray(b2b, np.float32).reshape(128, 1),
        "b2bt": np.ascontiguousarray(
            np.tile(np.asarray(b2b, np.float32).reshape(1, DF), (128, 1))
        ),
        "bms": _band_mask(r_self).astype(bf16),
        "cis": _cnt_inv(r_self),
    }
    if not share_tgt:
        common["bmt"] = _band_mask(r_tgt).astype(bf16)
        common["cit"] = _cnt_inv(r_tgt)

    in_maps = []
    for c in range(NCORES):
        x1t = np.ascontiguousarray(
            f1[c * NB : (c + 1) * NB].reshape(NLOC, DIN1).T.astype(bf16)
        )  # (256, 1024)
        # rotate feature2^T columns so this core's rows come first
        x2t = np.ascontiguousarray(
            np.concatenate(
                [x2t_full[:, c * NLOC :], x2t_full[:, : c * NLOC]], axis=1
            )
        )
        in_maps.append({**common, "x1t": x1t, "x2t": x2t})

    res = run_bass_kernel_spmd(
        nc,
        in_maps,
        core_ids=list(range(NCORES)),
        trace=trace,
        **(trace_kwargs or {}),
    )

    pos = np.empty(N, dtype=np.float64)
    se = np.empty(N, dtype=np.float64)
    for c in range(NCORES):
        # column t holds local rows [t*128, (t+1)*128) in partitions
        p = res.results[c]["pos_out"]  # (128, NT)
        s = res.results[c]["se_out"]
        pos[c * NLOC : (c + 1) * NLOC] = p.T.reshape(NLOC)
        se[c * NLOC : (c + 1) * NLOC] = s.T.reshape(NLOC)

    neg = np.log(se) - np.log(float(N))
    loss = np.mean(-pos + neg)
    return np.array(loss, dtype=np.float32), res


# revision 18
# speedup vs baseline: 1.1093x; 1.1093x over previous
"""Contrastive-learning loss kernel for Trainium2 (8 NeuronCores, Bass/Tile).

Problem (hardcoded shapes): B=16, L=512, DIN1=256, DIN2=192, DH=256, DF=128.
  emb1 = MLP_a(feature1); emb2 = MLP_b(feature2)          # (B, L, DF)
  positive = rowdot(f1, f2) + band-mean terms              # (N,)  N = B*L = 8192
  negative = logsumexp(f1 @ f2.T, axis=-1) - log N         # (N,)
  loss = mean(-positive + negative)

Sharding: data-parallel over B for embeddings/positives (2 batches per core);
the N x N negatives matrix is sharded row-wise. Each core computes the full
emb2 from a column-ROTATED copy of feature2 (its own batches first), so the
device program is identical across cores (pure SPMD, no partition-id): the
local rows are always columns [0, 1024) and logsumexp is invariant to column
order.

Schedule: the ScalarE exp (65536 elements/partition) is the hard floor
(~55us at 1 elem/cycle); everything else hides under it. All PSUM flows
through one 2-buffer (128, 2048) ring (8 banks), so every exp activation is
2048 wide with a fused row-sum (accum_out) and a fused per-row b2b bias
(exp(s + e1.b2b)); e2t itself stays bias-free. DMA descriptors are issued
from the GpSimd sequencer (cheap) in consumption order so MLP1 starts ~3us
and the first exp fires ~8us. MLP2 production, transposes, banded-positive
matmuls and the positive dot products are interleaved into PE/DVE/Pool slack
between negative chunks.

Outputs per core: pos_out (128, 8), se_out (128, 8) where column t holds
local rows [t*128, (t+1)*128). Host: loss = mean(-pos + log(se) - log N).
"""

import numpy as np

import concourse.bacc as bacc
import concourse.tile as tile
from concourse import mybir
from concourse.bass_utils import run_bass_kernel_spmd
from concourse.masks import make_identity

F32 = mybir.dt.float32
F32R = mybir.dt.float32r
BF16 = mybir.dt.bfloat16

B, L, DIN1, DIN2, DH, DF = 16, 512, 256, 192, 256, 128
N = B * L            # 8192 total rows
NCORES = 8
NB = B // NCORES     # 2 local batches per core
NLOC = NB * L        # 1024 local rows per core
NT = NLOC // 128     # 8 local row tiles
RING_FD = 2048       # ring tile free size (4 PSUM banks)
NGRP = N // RING_FD  # 4 negative column groups of 2048


def _build(share_tgt: bool):
    nc = bacc.Bacc("TRN2", target_bir_lowering=False, debug=False)

    x1t_d = nc.dram_tensor("x1t", [DIN1, NLOC], BF16, kind="ExternalInput")
    x2t_d = nc.dram_tensor("x2t", [DIN2, N], BF16, kind="ExternalInput")
    w1a_d = nc.dram_tensor("w1a", [DIN1, DH], BF16, kind="ExternalInput")
    w2a_d = nc.dram_tensor("w2a", [DH, DF], F32R, kind="ExternalInput")
    w1b_d = nc.dram_tensor("w1b", [DIN2, DH], BF16, kind="ExternalInput")
    w2b_d = nc.dram_tensor("w2b", [DH, DF], F32R, kind="ExternalInput")
    b1a_d = nc.dram_tensor("b1a", [128, 2], F32, kind="ExternalInput")
    b2a_d = nc.dram_tensor("b2a", [128, 1], F32, kind="ExternalInput")
    b1b_d = nc.dram_tensor("b1b", [128, 2], F32, kind="ExternalInput")
    b2b_d = nc.dram_tensor("b2b", [128, 1], F32, kind="ExternalInput")
    bms_d = nc.dram_tensor("bms", [L, L], BF16, kind="ExternalInput")
    cis_d = nc.dram_tensor("cis", [128, NT], F32, kind="ExternalInput")
    if not share_tgt:
        bmt_d = nc.dram_tensor("bmt", [L, L], BF16, kind="ExternalInput")
        cit_d = nc.dram_tensor("cit", [128, NT], F32, kind="ExternalInput")
    pos_d = nc.dram_tensor("pos_out", [128, NT], F32, kind="ExternalOutput")
    se_d = nc.dram_tensor("se_out", [128, NT], F32, kind="ExternalOutput")

    with tile.TileContext(nc) as tc:
        import contextlib

        with contextlib.ExitStack() as stack:
            const = stack.enter_context(tc.tile_pool(name="const", bufs=1))
            big = stack.enter_context(tc.tile_pool(name="big", bufs=1))
            h2pool = stack.enter_context(tc.tile_pool(name="h2pool", bufs=3))
            posp = stack.enter_context(tc.tile_pool(name="posp", bufs=2))
            psum = stack.enter_context(tc.tile_pool(name="psum", bufs=1, space="PSUM"))

            # ---- SBUF tiles -------------------------------------------------
            w1a = const.tile([128, 2, DH], BF16)
            x1t = big.tile([128, 2, NLOC], BF16)
            w1b_a = const.tile([128, DH], BF16)
            w1b_b = const.tile([64, DH], BF16)
            x2a = big.tile([128, N], BF16)
            x2b = big.tile([64, N], BF16)
            w2a = const.tile([128, 2, DF], F32R)
            w2b = const.tile([128, 2, DF], F32R)
            b1a = const.tile([128, 2], F32)
            b2a = const.tile([128, 1], F32)
            b1b = const.tile([128, 2], F32)
            b2b = const.tile([128, 1], F32)
            bms = const.tile([128, 4, L], BF16)
            cis = const.tile([128, NT], F32)
            if share_tgt:
                bmt, cit = bms, cis
            else:
                bmt = const.tile([128, 4, L], BF16)
                cit = const.tile([128, NT], F32)

            # ---- DMA issues split across 3 sequencers (sync/scalar/gpsimd)
            # so transfers start in parallel, emitted in consumption order.
            nc.sync.dma_start(
                out=w1a[:], in_=w1a_d.ap().rearrange("(t p) m -> p t m", p=128)
            )
            nc.sync.dma_start(
                out=x1t[:, :, 0:512],
                in_=x1t_d.ap().rearrange("(t p) c -> p t c", p=128)[:, :, 0:512],
            )
            nc.sync.dma_start(out=b1a[:], in_=b1a_d.ap())
            nc.sync.dma_start(
                out=x1t[:, :, 512:1024],
                in_=x1t_d.ap().rearrange("(t p) c -> p t c", p=128)[:, :, 512:1024],
            )
            nc.sync.dma_start(out=w1b_a[:], in_=w1b_d.ap()[0:128, :])
            nc.sync.dma_start(out=w1b_b[:], in_=w1b_d.ap()[128:DIN2, :])
            nc.sync.dma_start(
                out=w2a[:], in_=w2a_d.ap().rearrange("(t p) m -> p t m", p=128)
            )
            nc.sync.dma_start(out=b2a[:], in_=b2a_d.ap())

            nc.scalar.dma_start(out=x2a[:, 0:2048], in_=x2t_d.ap()[0:128, 0:2048])
            nc.scalar.dma_start(out=x2b[:, 0:2048], in_=x2t_d.ap()[128:DIN2, 0:2048])
            nc.scalar.dma_start(
                out=w2b[:], in_=w2b_d.ap().rearrange("(t p) m -> p t m", p=128)
            )
            for g in range(1, 4):
                cs = slice(g * 2048, (g + 1) * 2048)
                nc.scalar.dma_start(out=x2a[:, cs], in_=x2t_d.ap()[0:128, cs])
                nc.scalar.dma_start(out=x2b[:, cs], in_=x2t_d.ap()[128:DIN2, cs])

            ident = const.tile([128, 128], F32)
            make_identity(nc, ident[:])
            zr_l = const.tile([128, 128], BF16)
            nc.gpsimd.memset(zr_l[:], 0.0)
            zr_r = const.tile([128, 512], BF16)
            nc.gpsimd.memset(zr_r[:], 0.0)
            nc.gpsimd.dma_start(out=b1b[:], in_=b1b_d.ap())
            nc.gpsimd.dma_start(out=b2b[:], in_=b2b_d.ap())
            nc.gpsimd.dma_start(
                out=bms[:], in_=bms_d.ap().rearrange("(t p) j -> p t j", p=128)
            )
            nc.gpsimd.dma_start(out=cis[:], in_=cis_d.ap())
            if not share_tgt:
                nc.gpsimd.dma_start(
                    out=bmt[:], in_=bmt_d.ap().rearrange("(t p) j -> p t j", p=128)
                )
                nc.gpsimd.dma_start(out=cit[:], in_=cit_d.ap())

            e1t = big.tile([128, NLOC], F32R)
            e2t = big.tile([128, N], F32R)
            h1t = big.tile([128, 2, NLOC], F32R)
            e1nat = big.tile([128, NT, DF], BF16)
            e2nat = big.tile([128, NT, DF], BF16)
            w1nat = big.tile([128, NT, DF], F32)
            w2snat = big.tile([128, NT, DF], F32)
            w2tnat = w2snat if share_tgt else big.tile([128, NT, DF], F32)
            pos_all = big.tile([128, NT], F32)
            acc_all = big.tile([128, NT * NGRP], F32)
            se_all = big.tile([128, NT], F32)

            def ring(name):
                return psum.tile([128, RING_FD], F32, tag="ring", bufs=2, name=name)

            # ---- PE warm-up: ramp the clock while input DMAs stream --------
            wt = ring("warm")
            for i in range(4):
                nc.tensor.matmul(
                    wt[:, i * 512 : (i + 1) * 512], zr_l[:], zr_r[:],
                    start=True, stop=True,
                )

            # ---- MLP1: h1 = relu(W1a^T x1 + b1a); e1 = W2a^T h1 + b2a ------
            h1ps = ring("h1ps")  # [cc, mt, 512]
            for cc in range(2):
                for mt in range(2):
                    sl = slice(cc * 1024 + mt * 512, cc * 1024 + (mt + 1) * 512)
                    for kt in range(2):
                        nc.tensor.matmul(
                            h1ps[:, sl],
                            w1a[:, kt, mt * 128 : (mt + 1) * 128],
                            x1t[:, kt, cc * 512 : (cc + 1) * 512],
                            start=(kt == 0),
                            stop=(kt == 1),
                        )
            h1v = h1ps[:].rearrange("p (cc mt c) -> p cc mt c", cc=2, mt=2)
            for mt in range(2):
                nc.vector.tensor_scalar(
                    out=h1t[:, mt, :].rearrange("p (cc c) -> p cc c", cc=2),
                    in0=h1v[:, :, mt, :],
                    scalar1=b1a[:, mt : mt + 1],
                    scalar2=0.0,
                    op0=mybir.AluOpType.add,
                    op1=mybir.AluOpType.max,
                )
            e1ps = ring("e1ps")
            for cc in range(2):
                for kt in range(2):
                    nc.tensor.matmul(
                        e1ps[:, cc * 512 : (cc + 1) * 512],
                        w2a[:, kt, :],
                        h1t[:, kt, cc * 512 : (cc + 1) * 512],
                        start=(kt == 0),
                        stop=(kt == 1),
                    )
            nc.vector.tensor_scalar_add(
                out=e1t[:], in0=e1ps[:, 0:NLOC], scalar1=b2a[:]
            )

            # e1-side transposes (tokens -> partitions), needs only e1t
            tp1 = ring("tp1")
            for t in range(NT):
                nc.tensor.transpose(
                    tp1[:, t * 128 : (t + 1) * 128],
                    e1t[:, t * 128 : (t + 1) * 128].bitcast(F32),
                    ident[:],
                )
            nc.vector.tensor_copy(
                e1nat[:], tp1[:, 0 : NT * 128].rearrange("p (t f) -> p t f", t=NT)
            )


            # ---- MLP2 production: one ring turn per pair of 512-col chunks
            def mlp2_pair(cp):
                c0 = cp * 1024
                h2ps = ring(f"h2ps{cp}")  # [cc, mt, 512]
                for cc in range(2):
                    cols = slice(c0 + cc * 512, c0 + (cc + 1) * 512)
                    for mt in range(2):
                        sl = slice(cc * 1024 + mt * 512, cc * 1024 + (mt + 1) * 512)
                        msl = slice(mt * 128, (mt + 1) * 128)
                        nc.tensor.matmul(
                            h2ps[:, sl], w1b_a[:, msl], x2a[:, cols],
                            start=True, stop=False,
                        )
                        nc.tensor.matmul(
                            h2ps[:, sl], w1b_b[:, msl], x2b[:, cols],
                            start=False, stop=True,
                        )
                h2t = h2pool.tile([128, 2, 1024], F32R, tag="h2t", name=f"h2t{cp}")
                h2v = h2ps[:].rearrange("p (cc mt c) -> p cc mt c", cc=2, mt=2)
                for mt in range(2):
                    nc.vector.tensor_scalar(
                        out=h2t[:, mt, :].rearrange("p (cc c) -> p cc c", cc=2),
                        in0=h2v[:, :, mt, :],
                        scalar1=b1b[:, mt : mt + 1],
                        scalar2=0.0,
                        op0=mybir.AluOpType.add,
                        op1=mybir.AluOpType.max,
                    )
                e2ps = ring(f"e2ps{cp}")
                for cc in range(2):
                    for kt in range(2):
                        nc.tensor.matmul(
                            e2ps[:, cc * 512 : (cc + 1) * 512],
                            w2b[:, kt, :],
                            h2t[:, kt, cc * 512 : (cc + 1) * 512],
                            start=(kt == 0),
                            stop=(kt == 1),
                        )
                nc.vector.tensor_scalar_add(
                    out=e2t[:, c0 : c0 + 1024], in0=e2ps[:, 0:1024], scalar1=b2b[:]
                )

            mlp2_pair(0)  # local columns [0, 1024)

            tp2 = ring("tp2")
            for t in range(NT):
                nc.tensor.transpose(
                    tp2[:, t * 128 : (t + 1) * 128],
                    e2t[:, t * 128 : (t + 1) * 128].bitcast(F32),
                    ident[:],
                )
            nc.vector.tensor_copy(
                e2nat[:], tp2[:, 0 : NT * 128].rearrange("p (t f) -> p t f", t=NT)
            )

            mlp2_pair(1)

            # banded sums: one ring turn covers one batch (4 j-tiles, 512 cols)
            def band_batch(dst, bm, src, b):
                bp = ring(f"band{b}")
                for jt in range(4):
                    sl = slice(jt * 128, (jt + 1) * 128)
                    for mt in range(4):
                        nc.tensor.matmul(
                            bp[:, sl],
                            bm[:, mt, jt * 128 : (jt + 1) * 128],
                            src[:, 4 * b + mt, :],
                            start=(mt == 0),
                            stop=(mt == 3),
                        )
                nc.vector.tensor_copy(
                    dst[:, 4 * b : 4 * b + 4, :],
                    bp[:, 0:512].rearrange("p (s f) -> p s f", s=4),
                )

            # ---- negatives: one 2048-col exp per (row tile, col group) -----
            def neg_chunk(t, g):
                lhs = e1t[:, t * 128 : (t + 1) * 128]
                np_ps = ring(f"neg{t}_{g}")
                for i in range(RING_FD // 512):
                    c0 = g * RING_FD + i * 512
                    nc.tensor.matmul(
                        np_ps[:, i * 512 : (i + 1) * 512],
                        lhs,
                        e2t[:, c0 : c0 + 512],
                        start=True,
                        stop=True,
                    )
                idx = t * NGRP + g
                nc.scalar.activation(
                    out=np_ps[:],
                    in_=np_ps[:],
                    func=mybir.ActivationFunctionType.Exp,
                    accum_out=acc_all[:, idx : idx + 1],
                )

            # group 0 (cols 0:2048): pace one MLP2 pair per exp period so all
            # of e2 is produced while the PE stays dense (clock ramped)
            neg_chunk(0, 0)
            mlp2_pair(2)
            neg_chunk(1, 0)
            mlp2_pair(3)
            neg_chunk(2, 0)
            mlp2_pair(4)
            neg_chunk(3, 0)
            mlp2_pair(5)
            neg_chunk(4, 0)
            mlp2_pair(6)
            neg_chunk(5, 0)
            mlp2_pair(7)
            neg_chunk(6, 0)
            band_batch(w1nat, bms, e1nat, 0)
            neg_chunk(7, 0)
            band_batch(w1nat, bms, e1nat, 1)

            # group 1 with band turns as light filler
            neg_chunk(0, 1)
            band_batch(w2snat, bms, e2nat, 0)
            neg_chunk(1, 1)
            band_batch(w2snat, bms, e2nat, 1)
            neg_chunk(2, 1)
            if not share_tgt:
                band_batch(w2tnat, bmt, e2nat, 0)
            neg_chunk(3, 1)
            if not share_tgt:
                band_batch(w2tnat, bmt, e2nat, 1)
            for t in range(4, NT):
                neg_chunk(t, 1)

            # pos = rowdot(e1,e2loc) + band-mean terms (all 8 tiles at once)
            ga = posp.tile([128, NT, DF], F32, tag="posg")
            r1 = posp.tile([128, NT], F32, tag="post")
            r2 = posp.tile([128, NT], F32, tag="post")
            if share_tgt:
                nc.vector.tensor_add(ga[:], w1nat[:], w2snat[:])
                nc.vector.tensor_mul(ga[:], ga[:], e1nat[:])
            else:
                nc.vector.tensor_mul(ga[:], w1nat[:], e1nat[:])
            nc.vector.tensor_reduce(
                out=r1[:], in_=ga[:], axis=mybir.AxisListType.X, op=mybir.AluOpType.add
            )
            gb = posp.tile([128, NT, DF], F32, tag="posg")
            nc.vector.tensor_mul(gb[:], w2snat[:], e2nat[:])
            nc.vector.tensor_reduce(
                out=r2[:], in_=gb[:], axis=mybir.AxisListType.X, op=mybir.AluOpType.add
            )
            nc.vector.tensor_add(r1[:], r1[:], r2[:])
            nc.vector.tensor_mul(r1[:], r1[:], cis[:])
            if not share_tgt:
                gc = posp.tile([128, NT, DF], F32, tag="posg")
                nc.vector.tensor_mul(gc[:], w2tnat[:], e1nat[:])
                rt = posp.tile([128, NT], F32, tag="post")
                nc.vector.tensor_reduce(
                    out=rt[:], in_=gc[:], axis=mybir.AxisListType.X,
                    op=mybir.AluOpType.add,
                )
                nc.vector.tensor_mul(rt[:], rt[:], cit[:])
                nc.vector.tensor_add(r1[:], r1[:], rt[:])
            gd = posp.tile([128, NT, DF], BF16, tag="posgb")
            nc.vector.tensor_mul(gd[:], e1nat[:], e2nat[:])
            r3 = posp.tile([128, NT], F32, tag="post")
            nc.vector.tensor_reduce(
                out=r3[:], in_=gd[:], axis=mybir.AxisListType.X, op=mybir.AluOpType.add
            )
            nc.vector.tensor_add(pos_all[:], r1[:], r3[:])
            nc.sync.dma_start(out=pos_d.ap(), in_=pos_all[:])

            # groups 2..3
            for g in range(2, NGRP):
                for t in range(NT):
                    neg_chunk(t, g)

            nc.vector.tensor_reduce(
                out=se_all[:],
                in_=acc_all[:].rearrange("p (t g) -> p t g", t=NT),
                axis=mybir.AxisListType.X,
                op=mybir.AluOpType.add,
            )
            nc.sync.dma_start(out=se_d.ap(), in_=se_all[:])

    nc.compile()
    return nc


_BUILD_CACHE: dict = {}


def _get_nc(share_tgt: bool):
    if share_tgt not in _BUILD_CACHE:
        _BUILD_CACHE[share_tgt] = _build(share_tgt)
    return _BUILD_CACHE[share_tgt]


def _band_mask(r: int) -> np.ndarray:
    """mask[m, j] = 1 if |m-j| <= r (and inside [0,L)) else 0."""
    bm = np.zeros((L, L), dtype=np.float32)
    if r > 0:
        j = np.arange(L)
        lo = np.maximum(j - r, 0)
        hi = np.minimum(j + r + 1, L)
        m = np.arange(L)[:, None]
        bm = ((m >= lo[None, :]) & (m < hi[None, :])).astype(np.float32)
    return bm


def _cnt_inv(r: int) -> np.ndarray:
    """(128, NT) tile of 1/count(j) per local row (j = row mod L)."""
    j = np.arange(L)
    if r > 0:
        cnt = (np.minimum(j + r + 1, L) - np.maximum(j - r, 0)).astype(np.float64)
    else:
        cnt = np.ones(L)
    cinv = (1.0 / cnt).astype(np.float32)
    rows = (np.arange(NLOC) % L)
    return np.ascontiguousarray(cinv[rows].reshape(NT, 128).T)


def kernel(**inputs):
    loss, _ = _run(inputs, trace=False)
    return loss


def _run(inputs, trace=False, trace_kwargs=None):
    import ml_dtypes

    bf16 = ml_dtypes.bfloat16
    feature1 = inputs["feature1"]
    feature2 = inputs["feature2"]
    W1a, b1a, W2a, b2a = inputs["W1a"], inputs["b1a"], inputs["W2a"], inputs["b2a"]
    W1b, b1b, W2b, b2b = inputs["W1b"], inputs["b1b"], inputs["W2b"], inputs["b2b"]
    f1 = np.ascontiguousarray(np.asarray(feature1, dtype=np.float32))
    f2 = np.ascontiguousarray(np.asarray(feature2, dtype=np.float32))
    r_self = int(np.asarray(inputs["positive_range_self"]))
    r_tgt = int(np.asarray(inputs["positive_range_tgt"]))
    share_tgt = r_tgt == r_self

    nc = _get_nc(share_tgt)

    x2t_full = np.ascontiguousarray(f2.reshape(N, DIN2).T.astype(bf16))  # (192, 8192)
    common = {
        "w1a": np.ascontiguousarray(np.asarray(W1a, np.float32).astype(bf16)),
        "w2a": np.ascontiguousarray(np.asarray(W2a, np.float32)),
        "w1b": np.ascontiguousarray(np.asarray(W1b, np.float32).astype(bf16)),
        "w2b": np.ascontiguousarray(np.asarray(W2b, np.float32)),
        "b1a": np.ascontiguousarray(np.asarray(b1a, np.float32).reshape(2, 128).T),
        "b2a": np.asarray(b2a, np.float32).reshape(128, 1),
        "b1b": np.ascontiguousarray(np.asarray(b1b, np.float32).reshape(2, 128).T),
        "b2b": np.asarray(b2b, np.float32).reshape(128, 1),
        "bms": _band_mask(r_self).astype(bf16),
        "cis": _cnt_inv(r_self),
    }
    if not share_tgt:
        common["bmt"] = _band_mask(r_tgt).astype(bf16)
        common["cit"] = _cnt_inv(r_tgt)

    in_maps = []
    for c in range(NCORES):
        x1t = np.ascontiguousarray(
            f1[c * NB : (c + 1) * NB].reshape(NLOC, DIN1).T.astype(bf16)
        )  # (256, 1024)
        # rotate feature2^T columns so this core's rows come first
        x2t = np.ascontiguousarray(
            np.concatenate(
                [x2t_full[:, c * NLOC :], x2t_full[:, : c * NLOC]], axis=1
            )
        )
        in_maps.append({**common, "x1t": x1t, "x2t": x2t})

    res = run_bass_kernel_spmd(
        nc,
        in_maps,
        core_ids=list(range(NCORES)),
        trace=trace,
        **(trace_kwargs or {}),
    )

    pos = np.empty(N, dtype=np.float64)
    se = np.empty(N, dtype=np.float64)
    for c in range(NCORES):
        # column t holds local rows [t*128, (t+1)*128) in partitions
        p = res.results[c]["pos_out"]  # (128, NT)
        s = res.results[c]["se_out"]
        pos[c * NLOC : (c + 1) * NLOC] = p.T.reshape(NLOC)
        se[c * NLOC : (c + 1) * NLOC] = s.T.reshape(NLOC)

    neg = np.log(se) - np.log(float(N))
    loss = np.mean(-pos + neg)
    return np.array(loss, dtype=np.float32), res


# revision 19
# speedup vs baseline: 1.3969x; 1.2593x over previous
"""Contrastive-learning loss kernel for Trainium2 (8 NeuronCores, Bass/Tile).

Problem (hardcoded shapes): B=16, L=512, DIN1=256, DIN2=192, DH=256, DF=128.
  emb1 = MLP_a(feature1); emb2 = MLP_b(feature2)          # (B, L, DF)
  positive = rowdot(f1, f2) + band-mean terms              # (N,)  N = B*L = 8192
  negative = logsumexp(f1 @ f2.T, axis=-1) - log N         # (N,)
  loss = mean(-positive + negative)

Sharding: data-parallel over B for embeddings/positives (2 batches per core);
the N x N negatives matrix is sharded row-wise. Each core computes the full
emb2 from a column-ROTATED copy of feature2 (its own batches first), so the
device program is identical across cores (pure SPMD, no partition-id).

Schedule: the ScalarE exp stream (65536 elements/partition at 1 elem/cycle)
is the hard floor; the kernel is built to start it as early as possible and
never let it starve.
 - DMA issues are split across the sync/scalar/gpsimd sequencers in
   consumption order, so feature1 lands ~9us and MLP1 starts immediately.
 - Negatives use their own (128, 1024) x 2 PSUM ring (4 banks), decoupled
   from MLP2's PSUM (hps+sps, 4 banks), so e2 production and exp consumption
   only couple through the PE instruction queue. Production (MLP2 chunks,
   transposes, band matmuls) is paced a few matmuls per exp period.
 - Once all production PSUM users are done, the pools are swapped for a
   (128, 2048) x 2 ring (8 banks) and the remaining half of the negatives
   run as 2048-wide activations (halves the per-instruction overhead).

Outputs per core: pos_out (128, 8), se_out (128, 8) where column t holds
local rows [t*128, (t+1)*128). Host: loss = mean(-pos + log(se) - log N).
"""

import numpy as np

import concourse.bacc as bacc
import concourse.tile as tile
from concourse import mybir
from concourse.bass_utils import run_bass_kernel_spmd
from concourse.masks import make_identity

F32 = mybir.dt.float32
F32R = mybir.dt.float32r
BF16 = mybir.dt.bfloat16

B, L, DIN1, DIN2, DH, DF = 16, 512, 256, 192, 256, 128
N = B * L            # 8192 total rows
NCORES = 8
NB = B // NCORES     # 2 local batches per core
NLOC = NB * L        # 1024 local rows per core
NT = NLOC // 128     # 8 local row tiles
NGA = 4              # phase-A column groups of 1024 (cols 0:4096)
NGB = 2              # phase-B column groups of 2048 (cols 4096:8192)
NACC = NGA + NGB     # accumulator slots per row tile


def _build(share_tgt: bool):
    nc = bacc.Bacc("TRN2", target_bir_lowering=False, debug=False)

    x1t_d = nc.dram_tensor("x1t", [DIN1, NLOC], BF16, kind="ExternalInput")
    x2t_d = nc.dram_tensor("x2t", [DIN2, N], BF16, kind="ExternalInput")
    w1a_d = nc.dram_tensor("w1a", [DIN1, DH], BF16, kind="ExternalInput")
    w2a_d = nc.dram_tensor("w2a", [DH, DF], F32R, kind="ExternalInput")
    w1b_d = nc.dram_tensor("w1b", [DIN2, DH], BF16, kind="ExternalInput")
    w2b_d = nc.dram_tensor("w2b", [DH, DF], F32R, kind="ExternalInput")
    b1a_d = nc.dram_tensor("b1a", [128, 2], F32, kind="ExternalInput")
    b2a_d = nc.dram_tensor("b2a", [128, 1], F32, kind="ExternalInput")
    b1b_d = nc.dram_tensor("b1b", [128, 2], F32, kind="ExternalInput")
    b2b_d = nc.dram_tensor("b2b", [128, 1], F32, kind="ExternalInput")
    bms_d = nc.dram_tensor("bms", [L, L], BF16, kind="ExternalInput")
    cis_d = nc.dram_tensor("cis", [128, NT], F32, kind="ExternalInput")
    if not share_tgt:
        bmt_d = nc.dram_tensor("bmt", [L, L], BF16, kind="ExternalInput")
        cit_d = nc.dram_tensor("cit", [128, NT], F32, kind="ExternalInput")
    pos_d = nc.dram_tensor("pos_out", [128, NT], F32, kind="ExternalOutput")
    se_d = nc.dram_tensor("se_out", [128, NT], F32, kind="ExternalOutput")

    with tile.TileContext(nc) as tc:
        import contextlib

        with contextlib.ExitStack() as stack:
            const = stack.enter_context(tc.tile_pool(name="const", bufs=1))
            big = stack.enter_context(tc.tile_pool(name="big", bufs=1))
            h2pool = stack.enter_context(tc.tile_pool(name="h2pool", bufs=3))
            posp = stack.enter_context(tc.tile_pool(name="posp", bufs=2))

            # ---- SBUF tiles --------------------------------------------
            w1a = const.tile([128, 2, DH], BF16)
            x1t = big.tile([128, 2, NLOC], BF16)
            w1b_a = const.tile([128, DH], BF16)
            w1b_b = const.tile([64, DH], BF16)
            x2a = big.tile([128, N], BF16)
            x2b = big.tile([64, N], BF16)
            w2a = const.tile([128, 2, DF], F32R)
            w2b = const.tile([128, 2, DF], F32R)
            b1a = const.tile([128, 2], F32)
            b2a = const.tile([128, 1], F32)
            b1b = const.tile([128, 2], F32)
            b2b = const.tile([128, 1], F32)
            bms = const.tile([128, 4, L], BF16)
            cis = const.tile([128, NT], F32)
            if share_tgt:
                bmt, cit = bms, cis
            else:
                bmt = const.tile([128, 4, L], BF16)
                cit = const.tile([128, NT], F32)

            # ---- DMA issues split across 3 sequencers, consumption order
            nc.sync.dma_start(
                out=w1a[:], in_=w1a_d.ap().rearrange("(t p) m -> p t m", p=128)
            )
            for cc in range(2):
                nc.sync.dma_start(
                    out=x1t[:, :, cc * 512 : (cc + 1) * 512],
                    in_=x1t_d.ap().rearrange("(t p) c -> p t c", p=128)[
                        :, :, cc * 512 : (cc + 1) * 512
                    ],
                )
            nc.sync.dma_start(out=b1a[:], in_=b1a_d.ap())
            nc.sync.dma_start(
                out=w2a[:], in_=w2a_d.ap().rearrange("(t p) m -> p t m", p=128)
            )
            nc.sync.dma_start(out=b2a[:], in_=b2a_d.ap())
            nc.sync.dma_start(out=w1b_a[:], in_=w1b_d.ap()[0:128, :])
            nc.sync.dma_start(out=w1b_b[:], in_=w1b_d.ap()[128:DIN2, :])

            nc.scalar.dma_start(out=x2a[:, 0:2048], in_=x2t_d.ap()[0:128, 0:2048])
            nc.scalar.dma_start(out=x2b[:, 0:2048], in_=x2t_d.ap()[128:DIN2, 0:2048])
            nc.scalar.dma_start(
                out=w2b[:], in_=w2b_d.ap().rearrange("(t p) m -> p t m", p=128)
            )
            for g in range(1, 4):
                cs = slice(g * 2048, (g + 1) * 2048)
                nc.scalar.dma_start(out=x2a[:, cs], in_=x2t_d.ap()[0:128, cs])
                nc.scalar.dma_start(out=x2b[:, cs], in_=x2t_d.ap()[128:DIN2, cs])

            nc.gpsimd.dma_start(out=b1b[:], in_=b1b_d.ap())
            nc.gpsimd.dma_start(out=b2b[:], in_=b2b_d.ap())
            nc.gpsimd.dma_start(
                out=bms[:], in_=bms_d.ap().rearrange("(t p) j -> p t j", p=128)
            )
            nc.gpsimd.dma_start(out=cis[:], in_=cis_d.ap())
            if not share_tgt:
                nc.gpsimd.dma_start(
                    out=bmt[:], in_=bmt_d.ap().rearrange("(t p) j -> p t j", p=128)
                )
                nc.gpsimd.dma_start(out=cit[:], in_=cit_d.ap())

            ident = const.tile([128, 128], F32)
            make_identity(nc, ident[:])

            e1t = big.tile([128, NLOC], F32R)
            e2t = big.tile([128, N], F32R)
            h1t = big.tile([128, 2, NLOC], F32R)
            e1nat = big.tile([128, NT, DF], BF16)
            e2nat = big.tile([128, NT, DF], BF16)
            w1nat = big.tile([128, NT, DF], F32)
            w2snat = big.tile([128, NT, DF], F32)
            w2tnat = w2snat if share_tgt else big.tile([128, NT, DF], F32)
            pos_all = big.tile([128, NT], F32)
            acc_all = big.tile([128, NT * NACC], F32)
            se_all = big.tile([128, NT], F32)

            with contextlib.ExitStack() as stackA:
                psA = stackA.enter_context(
                    tc.tile_pool(name="psumA", bufs=1, space="PSUM")
                )

                # ---- MLP1: h1 = relu(W1a^T x1 + b1a); e1 = W2a^T h1 + b2a
                for cc in range(2):
                    cols = slice(cc * 512, (cc + 1) * 512)
                    h1ps = psA.tile([128, 2, 512], F32, tag="hps", bufs=1)
                    for mt in range(2):
                        for kt in range(2):
                            nc.tensor.matmul(
                                h1ps[:, mt, :],
                                w1a[:, kt, mt * 128 : (mt + 1) * 128],
                                x1t[:, kt, cols],
                                start=(kt == 0),
                                stop=(kt == 1),
                            )
                    for mt in range(2):
                        nc.vector.tensor_scalar(
                            out=h1t[:, mt, cols],
                            in0=h1ps[:, mt, :],
                            scalar1=b1a[:, mt : mt + 1],
                            scalar2=0.0,
                            op0=mybir.AluOpType.add,
                            op1=mybir.AluOpType.max,
                        )
                    e1ps = psA.tile([128, 512], F32, tag="sps", bufs=2)
                    for kt in range(2):
                        nc.tensor.matmul(
                            e1ps[:],
                            w2a[:, kt, :],
                            h1t[:, kt, cols],
                            start=(kt == 0),
                            stop=(kt == 1),
                        )
                    nc.vector.tensor_scalar_add(
                        out=e1t[:, cols], in0=e1ps[:], scalar1=b2a[:]
                    )

                # ---- MLP2 production, one 512-col chunk at a time ------
                def mlp2_chunk(ct):
                    cols = slice(ct * 512, (ct + 1) * 512)
                    h2ps = psA.tile(
                        [128, 2, 512], F32, tag="hps", bufs=1, name=f"h2ps{ct}"
                    )
                    for mt in range(2):
                        msl = slice(mt * 128, (mt + 1) * 128)
                        nc.tensor.matmul(
                            h2ps[:, mt, :], w1b_a[:, msl], x2a[:, cols],
                            start=True, stop=False,
                        )
                        nc.tensor.matmul(
                            h2ps[:, mt, :], w1b_b[:, msl], x2b[:, cols],
                            start=False, stop=True,
                        )
                    h2t = h2pool.tile([128, 2, 512], F32R, tag="h2t", name=f"h2t{ct}")
                    for mt in range(2):
                        nc.vector.tensor_scalar(
                            out=h2t[:, mt, :],
                            in0=h2ps[:, mt, :],
                            scalar1=b1b[:, mt : mt + 1],
                            scalar2=0.0,
                            op0=mybir.AluOpType.add,
                            op1=mybir.AluOpType.max,
                        )
                    e2ps = psA.tile([128, 512], F32, tag="sps", bufs=2, name=f"e2ps{ct}")
                    for kt in range(2):
                        nc.tensor.matmul(
                            e2ps[:], w2b[:, kt, :], h2t[:, kt, :],
                            start=(kt == 0), stop=(kt == 1),
                        )
                    nc.vector.tensor_scalar_add(
                        out=e2t[:, cols], in0=e2ps[:], scalar1=b2b[:]
                    )

                # 4 transposes (tokens->partitions) into one sps tile
                def tp_half(dst, srcT, half):
                    tp = psA.tile([128, 512], F32, tag="sps", bufs=2, name=f"tp{half}")
                    for k in range(4):
                        t = half * 4 + k
                        nc.tensor.transpose(
                            tp[:, k * 128 : (k + 1) * 128],
                            srcT[:, t * 128 : (t + 1) * 128].bitcast(F32),
                            ident[:],
                        )
                    nc.vector.tensor_copy(
                        dst[:, half * 4 : half * 4 + 4, :],
                        tp[:].rearrange("p (s f) -> p s f", s=4),
                    )

                # banded sums for one batch (4 j-tiles) into one sps tile
                def band_batch(dst, bm, src, b):
                    bp = psA.tile([128, 512], F32, tag="sps", bufs=2, name=f"band{b}")
                    for jt in range(4):
                        sl = slice(jt * 128, (jt + 1) * 128)
                        for mt in range(4):
                            nc.tensor.matmul(
                                bp[:, sl],
                                bm[:, mt, jt * 128 : (jt + 1) * 128],
                                src[:, 4 * b + mt, :],
                                start=(mt == 0),
                                stop=(mt == 3),
                            )
                    nc.vector.tensor_copy(
                        dst[:, 4 * b : 4 * b + 4, :],
                        bp[:].rearrange("p (s f) -> p s f", s=4),
                    )

                # phase-A negative chunk: (row tile t) x (1024 cols of group g)
                def neg1024(t, g):
                    lhs = e1t[:, t * 128 : (t + 1) * 128]
                    np_ps = psA.tile(
                        [128, 1024], F32, tag="negA", bufs=2, name=f"negA{t}_{g}"
                    )
                    for i in range(2):
                        c0 = g * 1024 + i * 512
                        nc.tensor.matmul(
                            np_ps[:, i * 512 : (i + 1) * 512],
                            lhs,
                            e2t[:, c0 : c0 + 512],
                            start=True,
                            stop=True,
                        )
                    nc.scalar.activation(
                        out=np_ps[:],
                        in_=np_ps[:],
                        func=mybir.ActivationFunctionType.Exp,
                        accum_out=acc_all[:, t * NACC + g : t * NACC + g + 1],
                    )

                mlp2_chunk(0)
                mlp2_chunk(1)

                # group 0 starts as soon as local e2 columns exist; remaining
                # production is paced into the exp stream's PE slack.
                fillers = [
                    lambda: mlp2_chunk(2),
                    lambda: mlp2_chunk(3),
                    lambda: tp_half(e1nat, e1t, 0),
                    lambda: tp_half(e1nat, e1t, 1),
                    lambda: mlp2_chunk(4),
                    lambda: mlp2_chunk(5),
                    lambda: tp_half(e2nat, e2t, 0),
                    lambda: tp_half(e2nat, e2t, 1),
                    lambda: mlp2_chunk(6),
                    lambda: mlp2_chunk(7),
                    lambda: mlp2_chunk(8),
                    lambda: mlp2_chunk(9),
                    lambda: mlp2_chunk(10),
                    lambda: mlp2_chunk(11),
                    lambda: mlp2_chunk(12),
                    lambda: mlp2_chunk(13),
                    lambda: mlp2_chunk(14),
                    lambda: mlp2_chunk(15),
                    lambda: band_batch(w1nat, bms, e1nat, 0),
                    lambda: band_batch(w1nat, bms, e1nat, 1),
                    lambda: band_batch(w2snat, bms, e2nat, 0),
                    lambda: band_batch(w2snat, bms, e2nat, 1),
                ]
                if not share_tgt:
                    fillers.append(lambda: band_batch(w2tnat, bmt, e2nat, 0))
                    fillers.append(lambda: band_batch(w2tnat, bmt, e2nat, 1))
                fi = 0
                for g in range(NGA):
                    for t in range(NT):
                        neg1024(t, g)
                        # ~2 production items per 3 exp periods
                        if (g * NT + t) % 3 != 2 and fi < len(fillers):
                            fillers[fi]()
                            fi += 1
                while fi < len(fillers):
                    fillers[fi]()
                    fi += 1

                # positives (DVE only): pos = rowdot(e1,e2loc) + band terms
                ga = posp.tile([128, NT, DF], F32, tag="posg")
                r1 = posp.tile([128, NT], F32, tag="post")
                r2 = posp.tile([128, NT], F32, tag="post")
                if share_tgt:
                    nc.vector.tensor_add(ga[:], w1nat[:], w2snat[:])
                    nc.vector.tensor_mul(ga[:], ga[:], e1nat[:])
                else:
                    nc.vector.tensor_mul(ga[:], w1nat[:], e1nat[:])
                nc.vector.tensor_reduce(
                    out=r1[:], in_=ga[:], axis=mybir.AxisListType.X,
                    op=mybir.AluOpType.add,
                )
                gb = posp.tile([128, NT, DF], F32, tag="posg")
                nc.vector.tensor_mul(gb[:], w2snat[:], e2nat[:])
                nc.vector.tensor_reduce(
                    out=r2[:], in_=gb[:], axis=mybir.AxisListType.X,
                    op=mybir.AluOpType.add,
                )
                nc.vector.tensor_add(r1[:], r1[:], r2[:])
                nc.vector.tensor_mul(r1[:], r1[:], cis[:])
                if not share_tgt:
                    gc = posp.tile([128, NT, DF], F32, tag="posg")
                    nc.vector.tensor_mul(gc[:], w2tnat[:], e1nat[:])
                    rt = posp.tile([128, NT], F32, tag="post")
                    nc.vector.tensor_reduce(
                        out=rt[:], in_=gc[:], axis=mybir.AxisListType.X,
                        op=mybir.AluOpType.add,
                    )
                    nc.vector.tensor_mul(rt[:], rt[:], cit[:])
                    nc.vector.tensor_add(r1[:], r1[:], rt[:])
                gd = posp.tile([128, NT, DF], BF16, tag="posgb")
                nc.vector.tensor_mul(gd[:], e1nat[:], e2nat[:])
                r3 = posp.tile([128, NT], F32, tag="post")
                nc.vector.tensor_reduce(
                    out=r3[:], in_=gd[:], axis=mybir.AxisListType.X,
                    op=mybir.AluOpType.add,
                )
                nc.vector.tensor_add(pos_all[:], r1[:], r3[:])
                nc.sync.dma_start(out=pos_d.ap(), in_=pos_all[:])

            # ---- phase B: all PSUM for 2048-wide exp chunks ------------
            with contextlib.ExitStack() as stackB:
                psB = stackB.enter_context(
                    tc.tile_pool(name="psumB", bufs=1, space="PSUM")
                )
                for G in range(NGB):
                    for t in range(NT):
                        lhs = e1t[:, t * 128 : (t + 1) * 128]
                        np_ps = psB.tile(
                            [128, 2048], F32, tag="negB", bufs=2, name=f"negB{t}_{G}"
                        )
                        for i in range(4):
                            c0 = NGA * 1024 + G * 2048 + i * 512
                            nc.tensor.matmul(
                                np_ps[:, i * 512 : (i + 1) * 512],
                                lhs,
                                e2t[:, c0 : c0 + 512],
                                start=True,
                                stop=True,
                            )
                        idx = t * NACC + NGA + G
                        nc.scalar.activation(
                            out=np_ps[:],
                            in_=np_ps[:],
                            func=mybir.ActivationFunctionType.Exp,
                            accum_out=acc_all[:, idx : idx + 1],
                        )

            nc.vector.tensor_reduce(
                out=se_all[:],
                in_=acc_all[:].rearrange("p (t g) -> p t g", t=NT),
                axis=mybir.AxisListType.X,
                op=mybir.AluOpType.add,
            )
            nc.sync.dma_start(out=se_d.ap(), in_=se_all[:])

    nc.compile()
    return nc


_BUILD_CACHE: dict = {}


def _get_nc(share_tgt: bool):
    if share_tgt not in _BUILD_CACHE:
        _BUILD_CACHE[share_tgt] = _build(share_tgt)
    return _BUILD_CACHE[share_tgt]


def _band_mask(r: int) -> np.ndarray:
    """mask[m, j] = 1 if |m-j| <= r (and inside [0,L)) else 0."""
    bm = np.zeros((L, L), dtype=np.float32)
    if r > 0:
        j = np.arange(L)
        lo = np.maximum(j - r, 0)
        hi = np.minimum(j + r + 1, L)
        m = np.arange(L)[:, None]
        bm = ((m >= lo[None, :]) & (m < hi[None, :])).astype(np.float32)
    return bm


def _cnt_inv(r: int) -> np.ndarray:
    """(128, NT) tile of 1/count(j) per local row (j = row mod L)."""
    j = np.arange(L)
    if r > 0:
        cnt = (np.minimum(j + r + 1, L) - np.maximum(j - r, 0)).astype(np.float64)
    else:
        cnt = np.ones(L)
    cinv = (1.0 / cnt).astype(np.float32)
    rows = (np.arange(NLOC) % L)
    return np.ascontiguousarray(cinv[rows].reshape(NT, 128).T)


def kernel(**inputs):
    loss, _ = _run(inputs, trace=False)
    return loss


def _run(inputs, trace=False, trace_kwargs=None):
    import ml_dtypes

    bf16 = ml_dtypes.bfloat16
    feature1 = inputs["feature1"]
    feature2 = inputs["feature2"]
    W1a, b1a, W2a, b2a = inputs["W1a"], inputs["b1a"], inputs["W2a"], inputs["b2a"]
    W1b, b1b, W2b, b2b = inputs["W1b"], inputs["b1b"], inputs["W2b"], inputs["b2b"]
    f1 = np.ascontiguousarray(np.asarray(feature1, dtype=np.float32))
    f2 = np.ascontiguousarray(np.asarray(feature2, dtype=np.float32))
    r_self = int(np.asarray(inputs["positive_range_self"]))
    r_tgt = int(np.asarray(inputs["positive_range_tgt"]))
    share_tgt = r_tgt == r_self

    nc = _get_nc(share_tgt)

    x2t_full = np.ascontiguousarray(f2.reshape(N, DIN2).T.astype(bf16))  # (192, 8192)
    common = {
        "w1a": np.ascontiguousarray(np.asarray(W1a, np.float32).astype(bf16)),
        "w2a": np.ascontiguousarray(np.asarray(W2a, np.float32)),
        "w1b": np.ascontiguousarray(np.asarray(W1b, np.float32).astype(bf16)),
        "w2b": np.ascontiguousarray(np.asarray(W2b, np.float32)),
        "b1a": np.ascontiguousarray(np.asarray(b1a, np.float32).reshape(2, 128).T),
        "b2a": np.asarray(b2a, np.float32).reshape(128, 1),
        "b1b": np.ascontiguousarray(np.asarray(b1b, np.float32).reshape(2, 128).T),
        "b2b": np.asarray(b2b, np.float32).reshape(128, 1),
        "bms": _band_mask(r_self).astype(bf16),
        "cis": _cnt_inv(r_self),
    }
    if not share_tgt:
        common["bmt"] = _band_mask(r_tgt).astype(bf16)
        common["cit"] = _cnt_inv(r_tgt)

    in_maps = []
    for c in range(NCORES):
        x1t = np.ascontiguousarray(
            f1[c * NB : (c + 1) * NB].reshape(NLOC, DIN1).T.astype(bf16)
        )  # (256, 1024)
        # rotate feature2^T columns so this core's rows come first
        x2t = np.ascontiguousarray(
            np.concatenate(
                [x2t_full[:, c * NLOC :], x2t_full[:, : c * NLOC]], axis=1
            )
        )
        in_maps.append({**common, "x1t": x1t, "x2t": x2t})

    res = run_bass_kernel_spmd(
        nc,
        in_maps,
        core_ids=list(range(NCORES)),
        trace=trace,
        **(trace_kwargs or {}),
    )

    pos = np.empty(N, dtype=np.float64)
    se = np.empty(N, dtype=np.float64)
    for c in range(NCORES):
        # column t holds local rows [t*128, (t+1)*128) in partitions
        p = res.results[c]["pos_out"]  # (128, NT)
        s = res.results[c]["se_out"]
        pos[c * NLOC : (c + 1) * NLOC] = p.T.reshape(NLOC)
        se[c * NLOC : (c + 1) * NLOC] = s.T.reshape(NLOC)

    neg = np.log(se) - np.log(float(N))
    loss = np.mean(-pos + neg)
    return np.array(loss, dtype=np.float32), res
